# revision 1
# baseline (speedup 1.0000x reference)
"""3-layer GAT on Trainium2, 8-core SPMD Bass kernel (v2).

Design (dst-partitioned, edge-gather, 128-wide blocks):
  - Nodes partitioned contiguously across 8 cores (6250/core); each core owns
    all edges whose dst is local, so segment-softmax/scatter-sum are local.
  - Per layer each core computes z_aug rows [z(D) | 1 | s] for its nodes
    (s = z.a_src, f32 packed in 2 fp16 slots), writes them to a DRAM table,
    AllGathers the full table, then per 128-dst block dma_gathers the z_aug
    rows of that block's edge sources (exact per-core counts via trailing -1
    indices), computes E = exp(leaky(s + t[dst])) for all 128 candidate dsts
    at once, masks with a one-hot (seg==iota) matrix, and reduces with PE
    matmuls: U[dst, :] = sum_e ex_e * z_aug[src_e]; the table's constant-1
    column yields the softmax denominator for free.
  - t = z.a_dst stays core-local ([npc] values, broadcast per block).
  - The next layer's z matmul is fused per block (transpose of the activated
    output feeds it immediately), so only the AllGather is a serial point.

Host preprocessing touches only src/dst (grouping, padding, int16 packing)
and weight layout; all float graph compute happens on device.
"""

import sys

import numpy as np

sys.path.insert(0, "/opt/trn_rl_repo")

import ml_dtypes  # noqa: E402

# --- problem constants (hardcoded) ---
N_NODES = 50000
N_EDGES = 800000
DIM_IN = 256
DIM_HID = 256
DIM_OUT = 128
N_CORES = 8

BLOCK = 128  # dst nodes per mask block
MAXCK = 8    # max chunks (x128 idx) per dma_gather call (ring capacity)
P = 128

NEG_SLOPE = 0.01


def _cdiv(a, b):
    return -(-a // b)


# ---------------------------------------------------------------------------
# host-side graph plan (pure index preprocessing)
# ---------------------------------------------------------------------------

def _pack_nodes(src, dst, n_nodes, n_cores, npc, H, nb):
    """Assign nodes to (core, block) bins so each bin's lo/hi in-edge counts
    fit a single <=1024-idx dma_gather call, and block loads are balanced
    across cores. Returns perm: position -> node id (position = core*npc +
    block*128 + slot). Pure index preprocessing."""
    lo_deg = np.bincount(dst[src < H], minlength=n_nodes)
    hi_deg = np.bincount(dst[src >= H], minlength=n_nodes)
    CAP = 1024
    nfull = (npc // BLOCK) * (n_cores // 2)
    lastsz = npc - (npc // BLOCK) * BLOCK
    halves = []
    for h0 in range(2):
        ids = np.arange(h0 * H, h0 * H + H)
        lo, hi = lo_deg[ids], hi_deg[ids]
        order = np.argsort(-(lo + hi))
        B = nfull + (n_cores // 2)
        sizes = np.array([BLOCK] * nfull + [lastsz] * (n_cores // 2))
        slo = np.zeros(B)
        shi = np.zeros(B)
        cnt = np.zeros(B, np.int64)
        binm = np.full(H, -1, np.int64)
        for j in order:
            l_, h_ = lo[j], hi[j]
            ov = (np.maximum(slo + l_ - CAP, 0)
                  + np.maximum(shi + h_ - CAP, 0))
            score = ov * 1e6 + np.maximum(slo + l_, shi + h_) \
                + (cnt >= sizes) * 1e18
            bb = int(np.argmin(score))
            binm[j] = bb
            slo[bb] += l_
            shi[bb] += h_
            cnt[bb] += 1
        # swap refinement: fix bins with slo or shi > CAP via 1-for-1 node
        # swaps (bin sizes preserved, so partial bins participate too)
        for _ in range(1000):
            over = np.where((slo > CAP) | (shi > CAP))[0]
            if len(over) == 0:
                break
            ob = int(over[0])
            members = np.where(binm == ob)[0]
            msort = members[np.argsort(-(lo[members] + hi[members]))]
            fixed = False
            for a in msort[:96]:
                la, ha = lo[a], hi[a]
                needl = slo[ob] - CAP
                needh = shi[ob] - CAP
                # candidate partners b anywhere: fixing ob, not breaking b's bin
                okb = ((lo <= la - needl)
                       & (hi <= ha - needh)
                       & (slo[binm] + la - lo <= CAP)
                       & (shi[binm] + ha - hi <= CAP)
                       & (binm != ob))
                w = np.where(okb)[0]
                if len(w):
                    bn_ = int(w[0])
                    ub = int(binm[bn_])
                    binm[a], binm[bn_] = ub, ob
                    slo[ob] += lo[bn_] - la
                    shi[ob] += hi[bn_] - ha
                    slo[ub] += la - lo[bn_]
                    shi[ub] += ha - hi[bn_]
                    fixed = True
                    break
            if not fixed:
                break
        halves.append((ids, binm, slo, shi, sizes))

    # assign bins to (core, position): sort full bins by load, group
    # similar ones at the same position; partial bins at the last position
    perm = np.zeros(n_nodes, np.int64)
    npos = npc // BLOCK  # full positions per core
    for h0, (ids, binm, slo, shi, sizes) in enumerate(halves):
        fullb = np.argsort(-(slo[:nfull] + shi[:nfull]))
        grid = fullb.reshape(npos, n_cores // 2)
        cores = range(h0 * (n_cores // 2), (h0 + 1) * (n_cores // 2))
        for p in range(npos):
            for ci, cc_ in enumerate(cores):
                bb = grid[p, ci]
                nodes = ids[np.where(binm == bb)[0]]
                pos0 = cc_ * npc + p * BLOCK
                perm[pos0: pos0 + len(nodes)] = nodes
        for ci, cc_ in enumerate(cores):
            bb = nfull + ci
            nodes = ids[np.where(binm == bb)[0]]
            pos0 = cc_ * npc + npos * BLOCK
            perm[pos0: pos0 + len(nodes)] = nodes
    return perm


def build_plan(src, dst, n_nodes, n_cores):
    src = np.asarray(src).astype(np.int64)
    dst = np.asarray(dst).astype(np.int64)
    assert n_nodes % n_cores == 0
    npc = n_nodes // n_cores
    H = (n_nodes + 1) // 2
    assert max(H, n_nodes - H) <= 32767

    nb = _cdiv(npc, BLOCK)
    perm = _pack_nodes(src, dst, n_nodes, n_cores, npc, H, nb)
    inv = np.zeros(n_nodes, np.int64)
    inv[perm] = np.arange(n_nodes)
    # rewrite the graph in position space (table rows = positions; the
    # within-half packing keeps position<H iff node<H)
    src = inv[src]
    dst = inv[dst]

    c = dst // npc
    loc = dst - c * npc
    b = loc // BLOCK
    sg = loc % BLOCK
    hf = (src >= H).astype(np.int64)

    key = (c * nb + b) * 2 + hf
    cnt = np.bincount(key, minlength=n_cores * nb * 2).reshape(n_cores, nb, 2)
    nmax = cnt.max(axis=0)  # [nb, 2] max edges per (block, half) across cores
    ck = _cdiv(nmax, 128)   # chunks per (block, half), shared across cores
    dead = ck.sum(axis=1) == 0
    ck[dead, 0] = 1  # keep >=1 chunk per block so U accumulation happens

    order = np.lexsort((src, hf, b, c))
    gsize = cnt.reshape(-1)
    gstart = np.zeros_like(gsize)
    gstart[1:] = np.cumsum(gsize)[:-1]

    # per-block layout offsets
    blocks = []
    seg_cols = 0
    i16lo = i16hi = 0
    for bb in range(nb):
        cklo, ckhi = int(ck[bb, 0]), int(ck[bb, 1])
        n16lo = _cdiv(int(nmax[bb, 0]), 16)
        n16hi = _cdiv(int(nmax[bb, 1]), 16)
        blocks.append(dict(
            cklo=cklo, ckhi=ckhi, cktot=cklo + ckhi,
            nlo=int(nmax[bb, 0]), nhi=int(nmax[bb, 1]),
            n16lo=n16lo, n16hi=n16hi,
            sgo=seg_cols, ilo=i16lo, ihi=i16hi,
        ))
        seg_cols += cklo + ckhi
        i16lo += n16lo
        i16hi += n16hi

    def wrap16(a):
        S = len(a) // 16
        w = a.reshape(S, 16).T
        return np.tile(w, (8, 1))

    idxlo = np.full((n_cores, P, i16lo), -1, np.int16)
    idxhi = np.full((n_cores, P, i16hi), -1, np.int16)
    seg = np.full((n_cores, P, seg_cols), 255.0, np.float32)

    for cc_ in range(n_cores):
        for bb in range(nb):
            bl = blocks[bb]
            for h, (ckh, n16, off, itab) in enumerate([
                (bl["cklo"], bl["n16lo"], bl["ilo"], idxlo),
                (bl["ckhi"], bl["n16hi"], bl["ihi"], idxhi),
            ]):
                n = int(cnt[cc_, bb, h])
                if n16 == 0:
                    continue
                s0 = gstart[(cc_ * nb + bb) * 2 + h]
                e = order[s0: s0 + n]
                # pad with idx 0 (gathers a valid row; seg=255 masks it out).
                # Trailing-negative trimming crashes the DGE ring bookkeeping,
                # so padded slots must stay gatherable.
                ids = np.zeros(n16 * 16, np.int64)
                ids[:n] = src[e] - (H if h else 0)
                itab[cc_, :, off: off + n16] = wrap16(ids)
                # seg values: slot j -> partition j%128, chunk j//128
                segs = np.full(ckh * 128, 255, np.int64)
                segs[:n] = sg[e]
                cs = bl["sgo"] + (bl["cklo"] if h else 0)
                seg[cc_, :, cs: cs + ckh] = segs.reshape(ckh, 128).T

    meta = dict(
        n_cores=n_cores, n_nodes=n_nodes, npc=npc, H=H, nb=nb,
        blocks=blocks, seg_cols=seg_cols, i16lo=i16lo, i16hi=i16hi,
        ckmax=max(bl["cktot"] for bl in blocks), perm=perm,
        n_calls=sum(_cdiv(bl["nlo"], 1024) + _cdiv(bl["nhi"], 1024)
                    for bl in blocks),
        n_desc=sum(bl["nlo"] + bl["nhi"] for bl in blocks),
    )
    per_core = dict(
        idxlo=idxlo, idxhi=idxhi,
        seg=seg.astype(ml_dtypes.bfloat16),
    )
    return meta, per_core


def const_inputs():
    iota = np.tile(np.arange(BLOCK, dtype=np.float32), (P, 1)).astype(
        ml_dtypes.bfloat16
    )
    id16 = np.eye(P, dtype=np.float16)
    id32 = np.eye(P, dtype=np.float32)
    return {"iota": iota, "id16": id16, "id32": id32}


def build_waug(W, A):
    d_out = W.shape[0]
    Wt = W.T.astype(np.float64)
    a_s = A[0, :d_out].astype(np.float64)
    a_d = A[0, d_out:].astype(np.float64)
    waug = np.concatenate([Wt, (Wt @ a_s)[:, None], (Wt @ a_d)[:, None]], axis=1)
    return waug.astype(np.float16)


# ---------------------------------------------------------------------------
# device program
# ---------------------------------------------------------------------------

def build_nc(meta, debug=False):
    import concourse.bacc as bacc
    import concourse.bass as bass
    import concourse.mybir as mybir
    import concourse.tile as tile
    from concourse.library_config import mlp

    dt = mybir.dt
    AP = bass.AP
    Alu = bass.mybir.AluOpType
    Act = bass.mybir.ActivationFunctionType

    npc = meta["npc"]
    H = meta["H"]
    N = meta["n_nodes"]
    nb = meta["nb"]
    n_cores = meta["n_cores"]
    blocks = meta["blocks"]
    CKMAX = meta["ckmax"]
    RB = 6  # g ring depth
    AGSPLIT = 32  # z-blocks in the first (early, overlapped) AllGather part

    # per-layer: (Dout, table stride in fp16 slots)
    DOUT = [DIM_HID, DIM_HID, DIM_OUT]
    STRIDE = [384, 384, 256]
    ELEM = 384  # gather element size (fp16 slots) for every layer

    nc = bacc.Bacc("TRN2", target_bir_lowering=False, debug=debug,
                   num_devices=n_cores, num_swdge_queues=4)

    h_in = nc.dram_tensor("h", [npc, DIM_IN], dt.float32, kind="ExternalInput")
    w_in = [
        nc.dram_tensor(f"w{l}", [(DIM_IN, DIM_HID, DIM_HID)[l], DOUT[l] + 2],
                       dt.float16, kind="ExternalInput")
        for l in range(3)
    ]
    ixlo_in = nc.dram_tensor("idxlo", [P, meta["i16lo"]], dt.int16,
                             kind="ExternalInput")
    ixhi_in = nc.dram_tensor("idxhi", [P, meta["i16hi"]], dt.int16,
                             kind="ExternalInput")
    seg_in = nc.dram_tensor("seg", [P, meta["seg_cols"]], dt.bfloat16,
                            kind="ExternalInput")
    iota_in = nc.dram_tensor("iota", [P, BLOCK], dt.bfloat16, kind="ExternalInput")
    id16_in = nc.dram_tensor("id16", [P, P], dt.float16, kind="ExternalInput")
    id32_in = nc.dram_tensor("id32", [P, P], dt.float32, kind="ExternalInput")
    out_t = nc.dram_tensor("out", [npc, DIM_OUT], dt.float32,
                           kind="ExternalOutput")

    agi = [nc.dram_tensor(f"agi{l}", [npc, STRIDE[l]], dt.float16)
           for l in range(3)]
    ago = [
        nc.dram_tensor(f"ago{l}", [npc * n_cores + (2 if l == 2 else 0),
                                   STRIDE[l]],
                       dt.float16, addr_space="Shared")
        for l in range(3)
    ]

    def bc_mid(ap2, n):
        return AP(ap2.tensor, ap2.offset, [ap2.ap[0], [0, n], ap2.ap[1]])

    def bc_last(ap2, n):
        return AP(ap2.tensor, ap2.offset, [ap2.ap[0], ap2.ap[1], [0, n]])

    with tile.TileContext(nc) as tc:
        import contextlib

        ctx = contextlib.ExitStack()
        with ctx:
            pers = ctx.enter_context(tc.tile_pool(name="pers", bufs=1))
            ph = ctx.enter_context(tc.tile_pool(name="ph", bufs=2))
            pe0 = ctx.enter_context(tc.tile_pool(name="pe0", bufs=2))
            pmsk = ctx.enter_context(tc.tile_pool(name="pmsk", bufs=2))
            psm = ctx.enter_context(tc.tile_pool(name="psm", bufs=2))
            pz = ctx.enter_context(tc.tile_pool(name="pz", bufs=2))
            psum_u = ctx.enter_context(tc.tile_pool(name="psu", bufs=2, space="PSUM"))
            psum_z = ctx.enter_context(tc.tile_pool(name="psz", bufs=2, space="PSUM"))
            psum_tr = ctx.enter_context(tc.tile_pool(name="pstr", bufs=2, space="PSUM"))
            psum_tt = ctx.enter_context(tc.tile_pool(name="pstt", bufs=1, space="PSUM"))
            psum_tb = ctx.enter_context(tc.tile_pool(name="pstb", bufs=1, space="PSUM"))

            nc.gpsimd.load_library(mlp)

            # ---- persistent state ----
            seg_sb = pers.tile([P, meta["seg_cols"]], dt.bfloat16, tag="seg", name="seg_sb")
            ixlo_sb = pers.tile([P, meta["i16lo"]], dt.int16, tag="ixlo", name="ixlo_sb")
            ixhi_sb = pers.tile([P, meta["i16hi"]], dt.int16, tag="ixhi", name="ixhi_sb")
            W_sb = [pers.tile([P, 2, DOUT[l] + 2], dt.float16, tag=f"w{l}", name=f"wsb{l}")
                    for l in range(3)]
            iota_sb = pers.tile([P, BLOCK], dt.bfloat16, tag="iota", name="iota_sb")
            id16 = pers.tile([P, P], dt.float16, tag="id16", name="id16")
            id32 = pers.tile([P, P], dt.float32, tag="id32", name="id32")
            trowt = [pers.tile([1, nb * BLOCK], dt.float16, tag=f"tr{i}", name=f"trowt{i}")
                     for i in range(2)]
            gb = [pers.tile([P, CKMAX, ELEM], dt.float16, tag=f"g{i}", name=f"gb{i}")
                  for i in range(RB)]
            ones1 = pers.tile([1, P], dt.float16, tag="ones1", name="ones1")

            nc.sync.dma_start(out=seg_sb[:], in_=seg_in[:, :])
            nc.sync.dma_start(out=ixlo_sb[:], in_=ixlo_in[:, :])
            nc.sync.dma_start(out=ixhi_sb[:], in_=ixhi_in[:, :])
            for l in range(3):
                nc.sync.dma_start(
                    out=W_sb[l][:],
                    in_=w_in[l].ap().rearrange("(k p) d -> p k d", p=P),
                )
            nc.sync.dma_start(out=iota_sb[:], in_=iota_in[:, :])
            nc.sync.dma_start(out=id16[:], in_=id16_in[:, :])
            nc.sync.dma_start(out=id32[:], in_=id32_in[:, :])
            for i in range(RB):
                nc.vector.memset(gb[i][:], 0.0)
            nc.vector.memset(ones1[:], 1.0)

            def zphase(b, stage, lnext, tci, first=False):
                """stage: [P, 256] fp16 transposed activations (x.T chunks).
                Computes z_aug for next-layer table, writes agi[lnext],
                stashes t column. Table row: [z(Dn) | 1 | pad | s_f32(2)]."""
                node0 = b * BLOCK
                bn = min(BLOCK, npc - node0)
                Dn = DOUT[lnext]
                zp = psum_z.tile([P, 258], dt.float32, tag="zp", name="zp")
                kch = stage.shape[1] // P
                for k in range(kch):
                    nc.tensor.matmul(
                        out=zp[:, : Dn + 2],
                        lhsT=stage[:, k * P: (k + 1) * P],
                        rhs=W_sb[lnext][:, k, : Dn + 2],
                        start=(k == 0),
                        stop=(k == kch - 1),
                    )
                asm = pz.tile([P, 384], dt.float16, tag="asm", name="asm")
                if first:
                    # zero the tail once per ring buffer so gathered rows
                    # never contain uninitialized DRAM
                    nc.vector.memset(asm[:, :], 0.0)
                nc.scalar.activation(asm[:bn, 0:Dn], zp[:bn, 0:Dn], Act.Copy)
                nc.vector.memset(asm[:bn, Dn: Dn + 1], 1.0)
                nc.vector.tensor_copy(
                    out=asm[:bn, Dn + 2: Dn + 4].bitcast(dt.float32),
                    in_=zp[:bn, Dn: Dn + 1],
                )
                # t column -> row b*128.. of t_rowT (PE transpose + copy)
                tcl = psm.tile([P, 1], dt.float32, tag="tcl", name="tcl")
                nc.vector.tensor_copy(out=tcl[:], in_=zp[:, Dn + 1: Dn + 2])
                tp = psum_tt.tile([1, P], dt.float32)
                nc.tensor.transpose(
                    out=tp[0:1, :P], in_=tcl[:, :], identity=id32[:, :],
                )
                nc.scalar.activation(
                    trowt[tci][0:1, node0: node0 + P], tp[0:1, :], Act.Copy
                )
                wcols = 384 if Dn == DIM_HID else Dn + 4
                nc.sync.dma_start(
                    out=agi[lnext][node0: node0 + bn, 0:wcols],
                    in_=asm[:bn, 0:wcols],
                )

            def allgather(l, r0, r1):
                S = STRIDE[l]
                a = ago[l].ap()
                out_ap = AP(a.tensor, r0 * S,
                            [[npc * S, n_cores], [S, r1 - r0], [1, S]])
                nc.gpsimd.collective_compute(
                    "AllGather",
                    bass.mybir.AluOpType.bypass,
                    replica_groups=[list(range(n_cores))],
                    ins=[agi[l].ap()[r0:r1, :].opt()],
                    outs=[out_ap.opt()],
                )

            # ---- layer-1 priming: h -> z_aug table + t ----
            for b in range(nb):
                node0 = b * BLOCK
                bn = min(BLOCK, npc - node0)
                hb = ph.tile([P, DIM_IN], dt.float32, tag="hb", name="hb")
                nc.sync.dma_start(out=hb[:bn], in_=h_in[node0: node0 + bn, :])
                h16 = ph.tile([P, DIM_IN], dt.float16, tag="h16", name="h16")
                nc.scalar.activation(h16[:bn], hb[:bn], Act.Copy)
                stage = ph.tile([P, DIM_IN], dt.float16, tag="st", name="st")
                for k in range(2):
                    ps = psum_tr.tile([P, P], dt.float16)
                    nc.tensor.transpose(
                        out=ps[:P, :bn],
                        in_=h16[:bn, k * P: (k + 1) * P],
                        identity=id16[:bn, :bn],
                    )
                    nc.vector.tensor_copy(
                        out=stage[:, k * P: (k + 1) * P], in_=ps[:, :]
                    )
                zphase(b, stage, 0, 0, first=(b < 2))
            allgather(0, 0, npc)

            # ---- 3 edge layers ----
            for l in range(3):
                Dn = DOUT[l]
                DU = Dn + 1
                SOFF = Dn + 2
                S = STRIDE[l]
                last = l == 2
                ag_ap = ago[l].ap()
                lo_tab = AP(ag_ap.tensor, 0, [[S, H], [1, ELEM]])
                hi_tab = AP(ag_ap.tensor, H * S, [[S, N - H], [1, ELEM]])

                if l == 2:
                    # s slots move to 129:131; stale layer-2 features there
                    # could bitcast to huge f32 -> exp overflow. Zero them.
                    for i in range(RB):
                        nc.vector.memset(gb[i][:, :, SOFF: SOFF + 2], 0.0)

                for b in range(nb):
                    bl = blocks[b]
                    node0 = b * BLOCK
                    bn = min(BLOCK, npc - node0)
                    cklo, cktot = bl["cklo"], bl["cktot"]
                    g = gb[b % RB]
                    for tab, n, i16off, c0 in [
                        (lo_tab, bl["nlo"], bl["ilo"], 0),
                        (hi_tab, bl["nhi"], bl["ihi"], cklo),
                    ]:
                        ixtab = ixlo_sb if c0 == 0 else ixhi_sb
                        done = 0
                        while done < n:
                            sub = min(n - done, MAXCK * 128)
                            nck = _cdiv(sub, 128)
                            o16 = i16off + done // 16
                            cs = c0 + done // 128
                            nc.gpsimd.dma_gather(
                                g[:, cs: cs + nck, :], tab,
                                ixtab[:, o16: o16 + _cdiv(sub, 16)],
                                sub, sub, ELEM, elem_step=S,
                                queue_num=b % 4,
                            )
                            done += sub
                    # tb[p, j] = t[dst j of block b] via PE outer product
                    tb = psum_tb.tile([P, BLOCK], dt.float32, tag="tb", name="tb")
                    nc.tensor.matmul(
                        out=tb[:, :],
                        lhsT=ones1[0:1, :],
                        rhs=trowt[l % 2][0:1, node0: node0 + BLOCK],
                        start=True, stop=True,
                    )
                    seg_v = seg_sb[:, bl["sgo"]: bl["sgo"] + cktot]
                    pt0 = pmsk.tile([P, CKMAX, BLOCK], dt.bfloat16, tag="pt0", name="pt0")
                    nc.vector.tensor_tensor(
                        out=pt0[:, 0:cktot, :],
                        in0=bc_last(seg_v, BLOCK),
                        in1=bc_mid(iota_sb[:], cktot),
                        op=Alu.is_equal,
                    )
                    sv = g[:, 0:cktot, SOFF: SOFF + 2].bitcast(dt.float32)
                    sv2 = AP(sv.tensor, sv.offset, [sv.ap[0], sv.ap[1]])
                    e0 = pe0.tile([P, CKMAX, BLOCK], dt.float32, tag="e0", name="e0")
                    nc.vector.tensor_tensor(
                        out=e0[:, 0:cktot, :],
                        in0=bc_last(sv2, BLOCK),
                        in1=bc_mid(tb[:], cktot),
                        op=Alu.add,
                    )
                    nc.vector.scalar_tensor_tensor(
                        out=e0[:, 0:cktot, :],
                        in0=e0[:, 0:cktot, :],
                        scalar=NEG_SLOPE,
                        in1=e0[:, 0:cktot, :],
                        op0=Alu.mult,
                        op1=Alu.max,
                    )
                    eb = pmsk.tile([P, CKMAX, BLOCK], dt.bfloat16, tag="eb", name="eb")
                    nc.scalar.activation(
                        eb[:, 0:cktot, :], e0[:, 0:cktot, :], Act.Exp
                    )
                    ptx = pmsk.tile([P, CKMAX, BLOCK], dt.bfloat16, tag="ptx", name="ptx")
                    nc.vector.tensor_tensor(
                        out=ptx[:, 0:cktot, :],
                        in0=pt0[:, 0:cktot, :],
                        in1=eb[:, 0:cktot, :],
                        op=Alu.mult,
                    )
                    U = psum_u.tile([P, 258], dt.float32, tag="U", name="U")
                    for k in range(cktot):
                        nc.tensor.matmul(
                            out=U[:, :DU],
                            lhsT=ptx[:, k, :],
                            rhs=g[:, k, 0:DU],
                            start=(k == 0),
                            stop=(k == cktot - 1),
                        )
                    den = psm.tile([P, 1], dt.float32, tag="den", name="den")
                    nc.vector.tensor_scalar(
                        out=den[:], in0=U[:, Dn: Dn + 1], scalar1=1e-9,
                        scalar2=None, op0=Alu.max,
                    )
                    rec = psm.tile([P, 1], dt.float32, tag="rec", name="rec")
                    nc.vector.reciprocal(rec[:], den[:])
                    xo = psm.tile([P, 256], dt.float32, tag="xo", name="xo")
                    nc.vector.tensor_scalar(
                        out=xo[:, 0:Dn], in0=U[:, 0:Dn], scalar1=rec[:],
                        scalar2=None, op0=Alu.mult,
                    )
                    if last:
                        nc.sync.dma_start(
                            out=out_t[node0: node0 + bn, :],
                            in_=xo[:bn, 0:DIM_OUT],
                        )
                        continue
                    a16 = psm.tile([P, 256], dt.float16, tag="a16", name="a16")
                    if l == 0:
                        nc.scalar.activation(a16[:, 0:Dn], xo[:, 0:Dn], Act.Tanh)
                    else:  # elu
                        mn = psm.tile([P, 256], dt.float32, tag="mn", name="mn")
                        nc.vector.tensor_scalar(
                            out=mn[:, 0:Dn], in0=xo[:, 0:Dn], scalar1=0.0,
                            scalar2=None, op0=Alu.min,
                        )
                        nc.scalar.activation(mn[:, 0:Dn], mn[:, 0:Dn], Act.Exp)
                        nc.vector.scalar_tensor_tensor(
                            out=mn[:, 0:Dn], in0=xo[:, 0:Dn], scalar=0.0,
                            in1=mn[:, 0:Dn], op0=Alu.max, op1=Alu.add,
                        )
                        nc.vector.tensor_scalar(
                            out=a16[:, 0:Dn], in0=mn[:, 0:Dn], scalar1=-1.0,
                            scalar2=None, op0=Alu.add,
                        )
                    stage = pz.tile([P, 256], dt.float16, tag="stg", name="stg")
                    for k in range(2):
                        ps = psum_tr.tile([P, P], dt.float16)
                        nc.tensor.transpose(
                            out=ps[:P, :P],
                            in_=a16[:, k * P: (k + 1) * P],
                            identity=id16[:, :],
                        )
                        nc.scalar.activation(
                            stage[:, k * P: (k + 1) * P], ps[:, :], Act.Copy
                        )
                    zphase(b, stage, l + 1, (l + 1) % 2)
                if not last:
                    allgather(l + 1, 0, npc)

    nc.compile()
    return nc


# ---------------------------------------------------------------------------
# entry point
# ---------------------------------------------------------------------------

_CACHE = {}


def _prepare(src, dst, n_nodes):
    key = (int(n_nodes), src.tobytes(), dst.tobytes())
    kh = hash(key)
    if kh not in _CACHE:
        meta, per_core = build_plan(src, dst, n_nodes, N_CORES)
        nc = build_nc(meta)
        _CACHE[kh] = (meta, per_core, nc)
    return _CACHE[kh]


def kernel(h, src, dst, n_nodes, W1, A1, W2, A2, W3, A3):
    from concourse.bass_utils import run_bass_kernel_spmd

    n_nodes = int(n_nodes)
    assert n_nodes == N_NODES
    meta, per_core, nc = _prepare(np.asarray(src), np.asarray(dst), n_nodes)
    npc = meta["npc"]

    w = [build_waug(W1, A1), build_waug(W2, A2), build_waug(W3, A3)]
    h = np.asarray(h, dtype=np.float32)[meta["perm"]]

    in_maps = []
    for c in range(N_CORES):
        in_maps.append(
            {
                "h": np.ascontiguousarray(h[c * npc: (c + 1) * npc]),
                "w0": w[0],
                "w1": w[1],
                "w2": w[2],
                "idxlo": per_core["idxlo"][c],
                "idxhi": per_core["idxhi"][c],
                "seg": per_core["seg"][c],
                **const_inputs(),
            }
        )
    res = run_bass_kernel_spmd(nc, in_maps, core_ids=list(range(N_CORES)))
    pos = np.concatenate([res.results[c]["out"] for c in range(N_CORES)], axis=0)
    out = np.empty((n_nodes, DIM_OUT), np.float32)
    out[meta["perm"]] = pos[:n_nodes]
    return out

